# revision 2
# baseline (speedup 1.0000x reference)
"""CenterLoss Trainium2 kernel (8 NeuronCores, data-parallel over batch).

Math: the reference builds the full [N, C] masked distance matrix, but only
the labeled entry of each row survives the mask, so

    loss = ( sum_i ||x_i - centers[labels_i]||^2  +  N*(C-1)*CLAMP_MIN ) / N

(the second term is the clamp applied to the zeroed-out entries).

v10 "bank-band" strategy: the host sorts each core's 2048 samples by label.
Each PSUM bank covers 4 tiles = 512 consecutive sorted samples whose labels
span < 128 centers (typical span ~35 of the core's ~135-label range), so a
single 128-row centers band per BANK serves all 4 of its tiles as the
matmul rhs (moving operand) - band switching is free; only the per-tile
one-hot lhsT pays a weight load. Packed fp8 input per core drops from 768KB
(v9 per-tile bands) to 576KB: [band0..band3 | (ohT_t | x_t) x 16].

Per tile, one PE matmul gathers the labeled centers into PSUM
(psum[s,d] = sum_w ohT[w,s]*band_k[w,d] = centers[label_s, d]), DVE
subtracts x (tensor_tensor, single PSUM operand), ACT squares + accumulates
per PSUM bank into acc[128, 4]. A final ones-matmul (reusing the framework
const-1.0 tile) reduces acc across partitions into psum[1,4], DVE
reduce_sum collapses it to [1,1], and a single-descriptor 4-byte DMA writes
the per-core partial (v9's [128,4] output DMA fanned into 128 tiny
descriptors). Host sums the 8 partials (the data-parallel all-reduce).

fp8 e4m3 input quantization costs ~6e-4 relative error (gate is 2e-2); the
PE gather and fp32 PSUM difference are exact given the quantized inputs.

Measured context (v9 baseline 18.4-19.1us): ~9.6us is fixed NEFF overhead
(preamble consts+barrier ~0.7us, end barrier ~0.8us, runtime postamble
~7.3-7.9us after the tile context - measured at 13.6us for a trivial
2-DMA kernel). The reducible middle is input DMA issue+stream, the
DVE->ACT tail, and the output DMA fan-out, which v10 attacks.

Host prep is layout/metadata only: sort + slice + dtype cast of inputs,
one-hot constants. All arithmetic on x and centers happens on device.
Falls back to v9 per-tile bands, then the v4 indirect-DMA gather kernel,
if a span check fails.
"""

import numpy as np

import concourse.bacc as bacc
import concourse.tile as tile
from concourse import bass, mybir
from concourse.bass_utils import run_bass_kernel_spmd

N, C, D = 16384, 1024, 128
N_CORES = 8
NS = N // N_CORES  # 2048 samples per core
P = 128
T = NS // P  # 16 tiles per core
W = 128  # band width
NB = 4  # psum banks
TPB = T // NB  # tiles per bank
CLAMP_MIN = 1e-12

USE_FP8 = True
BLK_DT = mybir.dt.float8e4 if USE_FP8 else mybir.dt.bfloat16

_cache = {}


def _blk_np_dtype():
    return mybir.dt.np(BLK_DT)


# ---------------------------------------------------------- v10: bank-band
# packed block layout (columns of the [128, BLK10_COLS] fp8 buffer):
#     [128*k : 128*(k+1)]          band for bank k (k=0..3)  (w -> d)
#     [512 + 256*t : +128]         ohT for tile t            (w -> s)
#     [512 + 256*t + 128 : +128]   x rows for tile t         (s -> d)
BAND_COLS = NB * P  # 512
TILE10_COLS = 2 * P  # 256
BLK10_COLS = BAND_COLS + T * TILE10_COLS  # 4608
# (chunk, engine): sync gets bands+odd banks, scalar even banks, so the two
# HWDGE rings generate descriptors in parallel and the tiny band chunk
# lands first.
DMA10_CHUNKS = (
    (0, BAND_COLS, "sync"),  # bands, 64KB
    (BAND_COLS, BAND_COLS + 4 * TILE10_COLS, "scalar"),  # bank0
    (BAND_COLS + 4 * TILE10_COLS, BAND_COLS + 8 * TILE10_COLS, "sync"),  # bank1
    (BAND_COLS + 8 * TILE10_COLS, BAND_COLS + 12 * TILE10_COLS, "scalar"),  # bank2
    (BAND_COLS + 12 * TILE10_COLS, BLK10_COLS, "sync"),  # bank3
)


def build_nc_bankband():
    nc = bacc.Bacc()
    blk = nc.declare_dram_parameter("blk", [P, BLK10_COLS], BLK_DT, isOutput=False)
    out = nc.declare_dram_parameter("out", [1, 1], mybir.dt.float32, isOutput=True)

    with tile.TileContext(nc) as tc:
        with (
            tc.tile_pool(name="data", bufs=1) as data,
            tc.tile_pool(name="small", bufs=1) as small,
            tc.tile_pool(name="psum", bufs=1, space="PSUM") as psump,
        ):
            sb = data.tile([P, BLK10_COLS], BLK_DT)
            scr0 = data.tile([P, TPB, P], mybir.dt.bfloat16)
            scr1 = data.tile([P, TPB, P], mybir.dt.bfloat16)
            scrs = (scr0, scr1, scr0, scr1)
            acc = data.tile([P, NB], mybir.dt.float32)

            for a, b, eng_name in DMA10_CHUNKS:
                eng = nc.scalar if eng_name == "scalar" else nc.sync
                eng.dma_start(out=sb[:, a:b], in_=blk[:, a:b])

            # tile-block view: [p, t, 256] over the post-band columns
            sb_t = sb[:, BAND_COLS:].rearrange("p (t c) -> p t c", c=TILE10_COLS)

            psums = []
            for k in range(NB):
                psum_k = psump.tile([P, TPB, P], mybir.dt.float32, tag=f"g{k}")
                psums.append(psum_k)
                band = sb[:, k * P : (k + 1) * P]
                for i in range(TPB):
                    t = k * TPB + i
                    base = BAND_COLS + t * TILE10_COLS
                    nc.tensor.matmul(
                        out=psum_k[:, i, :],
                        lhsT=sb[:, base : base + P],
                        rhs=band,
                        start=True, stop=True,
                    )
                # difference lands in an SBUF scratch (a PSUM-resident
                # difference measured 3-4us slower in v9: DVE/ACT/PE
                # contend on the PSUM ports), square-accumulate on ACT.
                d_sb = scrs[k]
                nc.vector.tensor_tensor(
                    out=d_sb[:, :, :],
                    in0=psum_k[:, :, :],
                    in1=sb_t[:, k * TPB : (k + 1) * TPB, P : 2 * P],
                    op=mybir.AluOpType.subtract,
                )
                nc.scalar.activation(
                    out=d_sb[:, :, :],
                    in_=d_sb[:, :, :],
                    func=mybir.ActivationFunctionType.Square,
                    accum_out=acc[:, k : k + 1],
                )

            # cross-partition reduce: ones^T @ acc -> psum[1, NB], then a
            # 4-element reduce to a single scalar so the output DMA is one
            # 4-byte descriptor instead of 128 16-byte ones.
            ones = nc.const_aps.aps[(mybir.dt.float32, 1.0)]
            psum_f = psump.tile([1, NB], mybir.dt.float32, tag="fin")
            nc.tensor.matmul(
                out=psum_f[:, :], lhsT=ones, rhs=acc[:, :], start=True, stop=True
            )
            res = small.tile([1, 1], mybir.dt.float32)
            nc.vector.reduce_sum(
                out=res[:1, :1], in_=psum_f[:1, :], axis=mybir.AxisListType.X
            )
            nc.sync.dma_start(out=out[:, :], in_=res[:1, :1])
    nc.compile()
    return nc


def prep_bankband_core(x_shard, labels_shard, centers_q):
    """Host layout prep for one core (v10). Returns in_map or None if a
    bank's 512 sorted samples span >= 128 distinct centers."""
    dt = _blk_np_dtype()
    order = np.argsort(labels_shard, kind="stable")
    ls = labels_shard[order].astype(np.int64)
    spb = TPB * P  # samples per bank = 512
    bases = np.minimum(ls[::spb][:NB], C - W)  # [NB]
    rel = ls.reshape(NB, spb) - bases[:, None]  # [NB, 512]
    if rel.min() < 0 or rel.max() >= W:
        return None
    xs = np.ascontiguousarray(x_shard[order]).astype(dt)
    iw = np.arange(W)[:, None]
    blk = np.zeros((P, BLK10_COLS), dtype=dt)
    for k in range(NB):
        blk[:, k * P : (k + 1) * P] = centers_q[bases[k] : bases[k] + W, :]
    rel_t = rel.reshape(T, P)
    for t in range(T):
        base = BAND_COLS + t * TILE10_COLS
        blk[:, base : base + P] = rel_t[t][None, :] == iw
        blk[:, base + P : base + 2 * P] = xs[t * P : (t + 1) * P, :]
    return {"blk": blk}


# -------------------------------------------------------------- v9: select
# (fallback) per-tile bands; see kernel_v9_backup.py docstring.
TILE_COLS = 3 * P  # 384
BLK_COLS = T * TILE_COLS  # 6144
DMA_CHUNKS = (4, 6, 6)  # tiles per input-DMA chunk
ACT_BANKS = (4, 4, 4, 4)  # tiles per PSUM bank / square-accumulate call


def build_nc_select():
    nc = bacc.Bacc()
    blk = nc.declare_dram_parameter("blk", [P, BLK_COLS], BLK_DT, isOutput=False)
    nout = len(ACT_BANKS)
    out = nc.declare_dram_parameter("out", [P, nout], mybir.dt.float32, isOutput=True)

    with tile.TileContext(nc) as tc:
        with (
            tc.tile_pool(name="data", bufs=1) as data,
            tc.tile_pool(name="psum", bufs=1, space="PSUM") as psump,
        ):
            sb = data.tile([P, BLK_COLS], BLK_DT)
            nbmax = max(ACT_BANKS)
            scr0 = data.tile([P, nbmax, P], mybir.dt.bfloat16)
            scr1 = data.tile([P, nbmax, P], mybir.dt.bfloat16)
            scrs = [(scr0, scr1)[k % 2] for k in range(len(ACT_BANKS))]
            acc = data.tile([P, nout], mybir.dt.float32)

            t0 = 0
            for k, nt in enumerate(DMA_CHUNKS):
                a, b = t0 * TILE_COLS, (t0 + nt) * TILE_COLS
                eng = nc.scalar if k % 2 else nc.sync
                eng.dma_start(out=sb[:, a:b], in_=blk[:, a:b])
                t0 += nt

            sb_t = sb[:, :].rearrange("p (t c) -> p t c", c=TILE_COLS)
            t = 0
            for k, nt in enumerate(ACT_BANKS):
                psum_k = psump.tile([P, nt, P], mybir.dt.float32, tag=f"g{k}")
                for i in range(nt):
                    base = (t + i) * TILE_COLS
                    nc.tensor.matmul(
                        out=psum_k[:, i, :],
                        lhsT=sb[:, base : base + P],
                        rhs=sb[:, base + P : base + 2 * P],
                        start=True, stop=True,
                    )
                d_sb = scrs[k]
                nc.vector.tensor_tensor(
                    out=d_sb[:, :nt, :],
                    in0=psum_k[:, :, :],
                    in1=sb_t[:, t : t + nt, 2 * P : 3 * P],
                    op=mybir.AluOpType.subtract,
                )
                nc.scalar.activation(
                    out=d_sb[:, :nt, :],
                    in_=d_sb[:, :nt, :],
                    func=mybir.ActivationFunctionType.Square,
                    accum_out=acc[:, k : k + 1],
                )
                t += nt
            nc.sync.dma_start(out=out[:, :], in_=acc[:, :])
    nc.compile()
    return nc


def prep_select_core(x_shard, labels_shard, centers_q):
    """Host layout prep for one core (v9). Returns in_map or None if a tile
    span exceeds the band width."""
    dt = _blk_np_dtype()
    order = np.argsort(labels_shard, kind="stable")
    ls = labels_shard[order].astype(np.int64)
    bases = np.minimum(ls[::P][:T], C - W)  # [T]
    rel = ls.reshape(T, P) - bases[:, None]  # [T, 128]
    if rel.min() < 0 or rel.max() >= W:
        return None
    xs = np.ascontiguousarray(x_shard[order]).astype(dt)
    iw = np.arange(W)[:, None]
    blk = np.zeros((P, BLK_COLS), dtype=dt)
    for t in range(T):
        base = t * TILE_COLS
        blk[:, base : base + P] = rel[t][None, :] == iw
        blk[:, base + P : base + 2 * P] = centers_q[bases[t] : bases[t] + W, :]
        blk[:, base + 2 * P : base + 3 * P] = xs[t * P : (t + 1) * P, :]
    return {"blk": blk}


# ------------------------------------------------- v4: indirect-DMA gather
def build_nc_gather(n_chunk=4, n_xdma=4):
    nc = bacc.Bacc()
    x = nc.declare_dram_parameter("x", [NS, D], mybir.dt.float32, isOutput=False)
    centers = nc.declare_dram_parameter(
        "centers", [C, D], mybir.dt.float32, isOutput=False
    )
    labels = nc.declare_dram_parameter("labels", [P, T], mybir.dt.int32, isOutput=False)
    out = nc.declare_dram_parameter("out", [1, 1], mybir.dt.float32, isOutput=True)

    x_t = x.rearrange("(t p) d -> p t d", p=P)
    tpc = T // n_chunk

    with tile.TileContext(nc) as tc:
        with (
            tc.tile_pool(name="data", bufs=1) as data,
            tc.tile_pool(name="small", bufs=1) as small,
            tc.tile_pool(name="psum", bufs=1, space="PSUM") as psump,
        ):
            x_sb = data.tile([P, T, D], mybir.dt.float32)
            g_sb = data.tile([P, T, D], mybir.dt.float32)
            d_sb = data.tile([P, T, D], mybir.dt.float32)
            i_sb = small.tile([P, T], mybir.dt.int32)
            acc = small.tile([P, n_chunk], mybir.dt.float32)
            ones = small.tile([P, 1], mybir.dt.float32)

            nc.vector.memset(ones[:], 1.0)
            nc.sync.dma_start(out=i_sb[:], in_=labels[:, :])
            tpx = T // n_xdma
            for j in range(n_xdma):
                xs = slice(j * tpx, (j + 1) * tpx)
                nc.sync.dma_start(out=x_sb[:, xs, :], in_=x_t[:, xs, :])
            for t in range(T):
                nc.gpsimd.indirect_dma_start(
                    out=g_sb[:, t, :],
                    out_offset=None,
                    in_=centers[:],
                    in_offset=bass.IndirectOffsetOnAxis(ap=i_sb[:, t : t + 1], axis=0),
                )
            for k in range(n_chunk):
                ts = slice(k * tpc, (k + 1) * tpc)
                nc.vector.tensor_tensor(
                    out=d_sb[:, ts, :],
                    in0=x_sb[:, ts, :],
                    in1=g_sb[:, ts, :],
                    op=mybir.AluOpType.subtract,
                )
                nc.scalar.activation(
                    out=d_sb[:, ts, :],
                    in_=d_sb[:, ts, :],
                    func=mybir.ActivationFunctionType.Square,
                    accum_out=acc[:, k : k + 1],
                )
            psum = psump.tile([1, n_chunk], mybir.dt.float32)
            nc.tensor.matmul(
                out=psum[:, :], lhsT=ones[:], rhs=acc[:], start=True, stop=True
            )
            res = small.tile([1, 1], mybir.dt.float32)
            nc.vector.reduce_sum(
                out=res[:1, :1], in_=psum[:1, :], axis=mybir.AxisListType.X
            )
            nc.sync.dma_start(out=out[:, :], in_=res[:1, :1])
    nc.compile()
    return nc


# ----------------------------------------------------------------- driver
def make_in_maps(x, centers, labels):
    """Returns (in_maps, which) where which is 'bankband', 'select' or
    'gather'."""
    x = np.ascontiguousarray(np.asarray(x, dtype=np.float32))
    centers = np.ascontiguousarray(np.asarray(centers, dtype=np.float32))
    labels = np.asarray(labels)

    centers_q = np.ascontiguousarray(centers.astype(_blk_np_dtype()))
    for which, prep in (("bankband", prep_bankband_core), ("select", prep_select_core)):
        in_maps = []
        for c in range(N_CORES):
            sl = slice(c * NS, (c + 1) * NS)
            m = prep(x[sl], labels[sl], centers_q)
            if m is None:
                break
            in_maps.append(m)
        else:
            return in_maps, which
    # fallback: indirect gather kernel
    in_maps = []
    for c in range(N_CORES):
        sl = slice(c * NS, (c + 1) * NS)
        in_maps.append(
            {
                "x": x[sl],
                "centers": centers,
                "labels": np.ascontiguousarray(
                    labels[sl].reshape(T, P).T.astype(np.int32)
                ),
            }
        )
    return in_maps, "gather"


_BUILDERS = {
    "bankband": build_nc_bankband,
    "select": build_nc_select,
    "gather": build_nc_gather,
}


def _get_nc(which):
    if which not in _cache:
        _cache[which] = _BUILDERS[which]()
    return _cache[which]


def finalize(results, which="bankband"):
    total = 0.0
    for c in range(N_CORES):
        o = np.asarray(results[c]["out"], dtype=np.float64)
        total += float(o.sum())
    total += N * (C - 1) * CLAMP_MIN
    return np.float32(total / N)


def kernel(x, centers, labels):
    in_maps, which = make_in_maps(x, centers, labels)
    nc = _get_nc(which)
    res = run_bass_kernel_spmd(nc, in_maps, core_ids=list(range(N_CORES)))
    return finalize(res.results, which)
